# revision 2
# baseline (speedup 1.0000x reference)
"""Self-contained distributed GAT kernel for 8 TRN2 NeuronCores (Bass/Tile).

Sharding: nodes (and incident edges, grouped by destination) across the
8 cores; weights replicated; per-layer feature tables exchanged via
AllGather; segment softmax/aggregation local per destination partition
in a rectangular [dst-row x edge-slot] layout filled by indexed DMA
gathers (4 parallel SWDGE queues, one per int16-addressable quarter).
Padded slots gather a reserved phantom "poison" row (s_src = -120) and
vanish in the softmax; aggregation is exp-weighted and normalized once
per group so each quarter's pipeline is independent.

kernel(**inputs) takes FULL inputs, returns (logits, probas) float32.
"""
import sys
import numpy as np

for _p in ('/opt/trn_rl_repo', '/root/.axon_site/_ro/trn_rl_repo'):
    if _p not in sys.path:
        sys.path.append(_p)

import concourse.bacc as bacc
from concourse import mybir, masks
from concourse.tile import TileContext
from concourse.bass_utils import run_bass_kernel_spmd
from contextlib import ExitStack

NCORES = 8
NQ = 4


def preprocess(edge_index, batch, N=100000, BLOCKS=98, NGRAPHS=256, GS=3):
    NPAD = NCORES * BLOCKS * 128
    SLAB = NPAD // NCORES
    QROWS = NPAD // 4
    GPC = NGRAPHS // NCORES
    NGRP = (BLOCKS + GS - 1) // GS
    loop = np.arange(N, dtype=np.int64)
    src = np.concatenate([loop, np.asarray(edge_index[0], dtype=np.int64)])
    dst = np.concatenate([loop, np.asarray(edge_index[1], dtype=np.int64)])
    batch = np.asarray(batch, dtype=np.int64)
    E = src.shape[0]

    deg = np.bincount(dst, minlength=N)   # includes self-loop

    # ---- pass 1: cores by total degree (snake), quarters FROZEN ----
    order = np.argsort(-deg, kind='stable')
    node_core = np.empty(N, np.int64)
    blk = np.arange(N) // 128
    s_, j_ = np.divmod(blk, NCORES)
    node_core[order] = np.where(s_ % 2 == 0, j_, NCORES - 1 - j_)
    node_quarter = node_core // 2
    degq = np.zeros((N, NQ), np.int64)
    np.add.at(degq, (dst, node_quarter[src]), 1)

    # ---- pass 2: per quarter-pair, rank-aligned (argmax, max, deg, q0, q1) ----
    mx = degq.max(axis=1)
    am = degq.argmax(axis=1)
    M64 = 64
    key = ((((am * M64 + mx) * M64 + deg.astype(np.int64)) * M64
            + degq[:, 0]) * M64 + degq[:, 1])
    tpos = np.empty(N, np.int64)
    full = (BLOCKS - 1) * 256     # pair ranks in full slots 0..BLOCKS-2
    cap = full + 254
    plists = []
    for p in range(4):
        nodes = np.where(node_quarter == p)[0]
        plists.append(list(nodes[np.argsort(-key[nodes], kind='stable')]))
    for p in range(4):                      # spill overflow to emptiest pair
        while len(plists[p]) > cap:
            tgt = min(range(4), key=lambda i: len(plists[i]))
            assert len(plists[tgt]) < cap
            plists[tgt].append(plists[p].pop())
    for p in range(4):
        o2 = np.array(plists[p], dtype=np.int64)
        assert len(o2) <= cap, "pair overflow (poison row reservation)"
        r = np.arange(len(o2))
        slot = r // 256
        sub = r % 256
        core = 2 * p + (sub // 128)
        row = sub % 128
        # last slot: reserve row 127 on BOTH cores (phantom poison rows)
        tail = r >= full
        st = r[tail] - full
        slot[tail] = BLOCKS - 1
        core[tail] = 2 * p + (st >= 127)
        row[tail] = np.where(st >= 127, st - 127, st)
        tpos[o2] = core * SLAB + slot * 128 + row

    # ---- geometry ----
    d_t = tpos[dst]
    d_core = d_t // SLAB
    d_rem = d_t - d_core * SLAB
    d_slot = d_rem // 128
    d_row = d_rem - d_slot * 128
    s_t = tpos[src]
    s_q = s_t // QROWS
    s_local = (s_t - s_q * QROWS).astype(np.int16)
    d_grp = d_slot // GS
    d_sloc = d_slot - d_grp * GS

    cnt = np.zeros((NCORES, BLOCKS, 128, NQ), np.int64)
    np.add.at(cnt, (d_core, d_slot, d_row, s_q), 1)
    slot_q_max = cnt.max(axis=(0, 2))                    # [BLOCKS, NQ]
    # group-uniform widths
    WG = np.zeros((NGRP, NQ), np.int64)
    GSL = np.zeros(NGRP, np.int64)                       # slots in group
    for g in range(NGRP):
        lo, hi = g * GS, min((g + 1) * GS, BLOCKS)
        GSL[g] = hi - lo
        WG[g] = slot_q_max[lo:hi].max(axis=0)
    WG = np.maximum(WG, 1)
    # per-group quarter region starts; group total width
    qg0 = np.zeros((NGRP, NQ + 1), np.int64)
    for g in range(NGRP):
        qg0[g, 1:] = np.cumsum(WG[g] * GSL[g])
    GW = qg0[:, -1]                                      # group widths
    g0 = np.concatenate([[0], np.cumsum(GW)])            # group col starts
    WTOT = int(g0[-1])

    # rank within (core, slot, row, quarter)
    kk = ((d_core * BLOCKS + d_slot) * 128 + d_row) * NQ + s_q
    eorder = np.argsort(kk, kind='stable')
    ks = kk[eorder]
    first = np.concatenate([[True], ks[1:] != ks[:-1]])
    runstart = np.maximum.accumulate(np.where(first, np.arange(E), 0))
    rank = np.arange(E) - runstart

    lidx = np.full((NCORES, 128, WTOT), SLAB - 1, np.int16)  # poison row
    padmask = np.ones((NCORES, 128, WTOT), bool)
    eo = eorder
    col = (g0[d_grp[eo]] + qg0[d_grp[eo], s_q[eo]]
           + d_sloc[eo] * WG[d_grp[eo], s_q[eo]] + rank)
    lidx[d_core[eo], d_row[eo], col] = s_local[eo]
    padmask[d_core[eo], d_row[eo], col] = False
    maskneg = np.where(padmask, np.float32(-60.0), np.float32(0.0))
    assert (~padmask).sum() == E

    node_at = np.full((NCORES, SLAB), -1, np.int64)
    lp = tpos - (tpos // SLAB) * SLAB
    node_at[tpos // SLAB, lp] = np.arange(N)
    assert (node_at[:, SLAB - 1] == -1).all(), "last slab row must be phantom (poison row)"


    # ---- pooling: graph g -> (core, grow); its nodes on partitions
    # 4*grow + subrow (subrow = within-(graph,quarter) rank % 4) ----
    gsize = np.bincount(batch, minlength=NGRAPHS)
    gorder = np.argsort(-gsize, kind='stable')
    pool_core = np.empty(NGRAPHS, np.int64)
    pool_row = np.empty(NGRAPHS, np.int64)
    for i, g in enumerate(gorder):
        r, j = divmod(i, NCORES)
        pool_core[g] = j if r % 2 == 0 else NCORES - 1 - j
        pool_row[g] = r

    keyp = (pool_core[batch] * GPC + pool_row[batch]) * NQ + (tpos // QROWS)
    porder = np.argsort(keyp, kind='stable')
    kp = keyp[porder]
    firstp = np.concatenate([[True], kp[1:] != kp[:-1]])
    runstart = np.maximum.accumulate(np.where(firstp, np.arange(N), 0))
    rankp = np.arange(N) - runstart
    subrow = rankp % 4
    jcol = rankp // 4
    # per-quarter width = max over (core, graph) of ceil(count/4)
    pq = np.zeros((NCORES, GPC, NQ), np.int64)
    np.add.at(pq, (pool_core[batch], pool_row[batch], tpos // QROWS), 1)
    PWQS = np.maximum((pq + 3) // 4, 1).max(axis=(0, 1))   # [NQ]
    pq0 = np.concatenate([[0], np.cumsum(PWQS)])
    WPS = int(pq0[-1])

    pool_lidx = np.zeros((NCORES, 128, WPS), np.int16)
    pool_pad = np.ones((NCORES, 128, WPS), bool)
    pc = pool_core[batch][porder]
    pr = pool_row[batch][porder]
    ppart = pr * 4 + subrow
    pcol = pq0[(tpos // QROWS)[porder]] + jcol
    pool_lidx[pc, ppart, pcol] = (tpos - (tpos // QROWS) * QROWS)[porder].astype(np.int16)
    pool_pad[pc, ppart, pcol] = False
    pool_maskneg = np.where(pool_pad, np.float32(-1e30), np.float32(0.0))
    out_graph = np.empty((NCORES, GPC), np.int64)
    out_graph[pool_core, pool_row] = np.arange(NGRAPHS)

    return dict(
        tpos=tpos, node_at=node_at, BLOCKS=BLOCKS,
        lidx=lidx, maskneg=maskneg, WG=WG, GSL=GSL, qg0=qg0, g0=g0, WTOT=WTOT,
        pool_lidx=pool_lidx, pool_maskneg=pool_maskneg, PWQS=PWQS, pq0=pq0,
        WPS=WPS, out_graph=out_graph, deg=deg,
    )


def wrap_idx(vals):
    """[..., n] int16, n % 16 == 0: idx i -> [i%16, i//16], replicated x8 to
    128 partitions -> [..., 128, n/16]."""
    sh = vals.shape[:-1]
    n = vals.shape[-1]
    assert n % 16 == 0
    w = vals.reshape(*sh, n // 16, 16)
    w = np.swapaxes(w, -1, -2)
    w = np.broadcast_to(w[..., None, :, :], (*sh, 8, 16, n // 16))
    return w.reshape(*sh, 128, n // 16).copy()


def expand_a(a):
    heads, ch = a.shape
    A = np.zeros((heads * ch, heads), np.float32)
    for h in range(heads):
        A[h * ch:(h + 1) * ch, h] = a[h]
    return A




FP = mybir.dt.float32
BF = mybir.dt.bfloat16
I16 = mybir.dt.int16
ALU = mybir.AluOpType
ACTF = mybir.ActivationFunctionType
AX = mybir.AxisListType

NCORES = 8
NQ = 4
NL = 4
GPC = 32
NEG = 0.2
NEG_OUT = 0.01
HEADS = (4, 4, 4, 1)


def build(nc, geom):
    BLOCKS = int(geom["BLOCKS"])
    NP_ = NCORES * BLOCKS * 128
    SLAB = NP_ // NCORES
    QROWS = NP_ // NQ
    WG = np.asarray(geom["WG"])            # [NGRP, NQ]
    GSL = np.asarray(geom["GSL"])          # [NGRP]
    qg0 = np.asarray(geom["qg0"])          # [NGRP, NQ+1]
    g0 = np.asarray(geom["g0"])            # [NGRP+1]
    WTOT = int(geom["WTOT"])
    PWQS = np.asarray(geom["PWQS"])        # [NQ]
    pq0 = np.asarray(geom["pq0"])          # [NQ+1]
    WPS = int(geom["WPS"])
    NGRP = len(GSL)

    # ---------------- I/O ----------------
    xT = nc.declare_dram_parameter("xT", [128, SLAB], FP, isOutput=False)
    idx_in = nc.declare_dram_parameter("idx", [128, 8 * WTOT], I16, isOutput=False)
    pidx_in = nc.declare_dram_parameter("pool_idx", [128, 8 * WPS], I16, isOutput=False)
    pmask_in = nc.declare_dram_parameter("pool_maskneg", [128, WPS], FP, isOutput=False)
    wext_in = [
        nc.declare_dram_parameter(f"wext{l}", [128 if l == 0 else 64, 64 + 2 * HEADS[l]],
                                  FP, isOutput=False)
        for l in range(NL)
    ]
    bias_in = nc.declare_dram_parameter("bias", [128, NL, 64], FP, isOutput=False)
    fcW_in = nc.declare_dram_parameter("fcW", [64, 2], FP, isOutput=False)
    fcb_in = nc.declare_dram_parameter("fcb", [GPC, 2], FP, isOutput=False)
    logits_out = nc.declare_dram_parameter("logits", [GPC, 2], FP, isOutput=True)
    probas_out = nc.declare_dram_parameter("probas", [GPC, 2], FP, isOutput=True)

    with TileContext(nc) as tc, ExitStack() as ex:
        dram = ex.enter_context(tc.tile_pool(name="dram", bufs=1, space="DRAM"))
        tables = [dram.tile([NP_, 128], I16, addr_space="Shared", name=f"table{l}")
                  for l in range(NL + 1)]
        slabs = [dram.tile([SLAB, 128], I16, name=f"slab{l}") for l in range(NL + 1)]

        cpool = ex.enter_context(tc.tile_pool(name="const", bufs=1))
        gpools = [ex.enter_context(tc.tile_pool(name=f"gath{q}", bufs=3)) for q in range(NQ)]
        wpool = ex.enter_context(tc.tile_pool(name="wrk", bufs=3))
        xpool = ex.enter_context(tc.tile_pool(name="xin", bufs=3))
        rpool = ex.enter_context(tc.tile_pool(name="rows", bufs=3))
        ppool = ex.enter_context(tc.tile_pool(name="psum", bufs=4, space="PSUM"))
        ppool2 = ex.enter_context(tc.tile_pool(name="psum2", bufs=2, space="PSUM"))

        # ---- constants resident in SBUF ----
        ipool = ex.enter_context(tc.tile_pool(name="idxs", bufs=4))
        wext = []
        for l in range(NL):
            t = cpool.tile([128 if l == 0 else 64, 64 + 2 * HEADS[l]], FP,
                           name=f"wext_sb{l}")
            nc.sync.dma_start(t[:], wext_in[l][:])
            wext.append(t)
        bias_sb = cpool.tile([128, NL, 64], FP)
        nc.sync.dma_start(bias_sb[:], bias_in[:])
        ident = cpool.tile([128, 128], FP)
        masks.make_identity(nc, ident[:])
        poison_t = cpool.tile([1, 4], FP)
        nc.vector.memset(poison_t[:], -120.0)
        sdst_self = [cpool.tile([128, BLOCKS, 4], FP, name=f"sdst{i}") for i in range(2)]

        def matmul_to_row(l, s, lhsT_ap):
            """h_ext = lhsT.T @ wext[l] -> row [128,128] bf16 -> slab[l]; also
            stashes s_dst into sdst_self[l % 2]."""
            H = HEADS[l]
            pm = ppool.tile([128, 64 + 2 * H], FP, tag="mm", name=f"mm_{l}_{s}")
            nc.tensor.matmul(pm[:], lhsT_ap, wext[l][:], start=True, stop=True)
            row = rpool.tile([128, 128], I16, tag="row", name=f"row_{l}_{s}")
            nc.scalar.copy(row[:].bitcast(BF)[:, 0:64], pm[:, 0:64])
            rf = row[:].bitcast(FP)
            nc.vector.tensor_copy(rf[:, 32:32 + 2 * H], pm[:, 64:64 + 2 * H])
            # s_dst from the SBUF row (avoids a second PSUM read per block)
            nc.vector.tensor_copy(sdst_self[l % 2][:, s, 0:H], rf[:, 32 + H:32 + 2 * H])
            nc.vector.memset(rf[:, 32 + 2 * H:64], 0.0)
            nc.sync.dma_start(slabs[l][s * 128:(s + 1) * 128, :], row[:])
            if s == BLOCKS - 1:
                # poison row: phantom last slab row's s_src <- -120 so padded
                # gather slots vanish in the softmax (exp(leaky(-120+sdst))~0)
                nc.sync.dma_start(slabs[l][SLAB - 1:SLAB, 64:72],
                                  poison_t[:].bitcast(I16))

        # ---- layer-0 matmul phase ----
        for s in range(BLOCKS):
            xt = xpool.tile([128, 128], FP, tag="xt", name=f"xt0_{s}")
            nc.sync.dma_start(xt[:], xT[:, s * 128:(s + 1) * 128])
            matmul_to_row(0, s, xt[:])

        # ---- layers ----
        for l in range(NL):
            nc.gpsimd.collective_compute(
                "AllGather", ALU.bypass,
                ins=[slabs[l][:].opt()],
                outs=[tables[l][:].opt()],
                replica_groups=[list(range(NCORES))],
            )
            H = HEADS[l]
            ch = 64 // H
            for g in range(NGRP):
                lo = int(sum(GSL[:g]))
                nsl = int(GSL[g])
                GWg = int(qg0[g][NQ])
                itile = ipool.tile([128, 8 * GWg], I16, tag="idx",
                                   name=f"idx_{l}_{g}")
                nc.sync.dma_start(itile[:], idx_in[:, 8 * int(g0[g]):
                                               8 * int(g0[g] + GWg)])
                Gq = []
                for q in range(NQ):
                    wq = int(WG[g][q])
                    ncols = nsl * wq
                    t = gpools[q].tile([128, ncols, 128], I16, tag=f"G{q}",
                                       name=f"G_{l}_{g}_{q}")
                    r0 = int(qg0[g][q])
                    nc.gpsimd.dma_gather(
                        t[:], tables[l][q * QROWS:(q + 1) * QROWS, :],
                        itile[:, 8 * r0: 8 * (r0 + ncols)],
                        128 * ncols, 128 * ncols, 128,
                        single_packet=False, queue_num=q)
                    Gq.append(t)
                den = wpool.tile([128, nsl, 4], FP, tag="den", name=f"den_{l}_{g}")
                outg = wpool.tile([128, nsl, 64], FP, tag="outg", name=f"og_{l}_{g}")
                for q in range(NQ):
                    wq = int(WG[g][q])
                    Gf = Gq[q][:].bitcast(FP).rearrange("p (s j) e -> p s j e", s=nsl)
                    ssrc = Gf[:, :, :, 32:32 + H]
                    e = wpool.tile([128, nsl, wq, H], FP, tag=f"e{q}",
                                   name=f"e_{l}_{g}_{q}")
                    nc.vector.tensor_tensor(
                        e[:], ssrc,
                        sdst_self[l % 2][:, lo:lo + nsl, 0:H]
                        .unsqueeze(2).broadcast_to([128, nsl, wq, H]),
                        ALU.add)
                    nc.vector.scalar_tensor_tensor(e[:], e[:], NEG, e[:],
                                                   ALU.mult, ALU.max)
                    ext = wpool.tile([128, nsl, wq, H], BF, tag=f"ex{q}",
                                     name=f"ex_{l}_{g}_{q}")
                    nc.scalar.activation(ext[:], e[:], ACTF.Exp)
                    dq = wpool.tile([128, nsl, 4], FP, tag=f"dq{q}",
                                    name=f"dq_{l}_{g}_{q}")
                    nc.vector.tensor_reduce(
                        dq[:, :, 0:H], ext[:].rearrange("p s j h -> p s h j"),
                        axis=AX.X, op=ALU.add)
                    if q == 0:
                        nc.vector.tensor_copy(den[:, :, 0:H], dq[:, :, 0:H])
                    else:
                        nc.vector.tensor_tensor(den[:, :, 0:H], den[:, :, 0:H],
                                                dq[:, :, 0:H], ALU.add)
                    # unnormalized weighted aggregation (releases Gq early)
                    wt = wpool.tile([128, nsl, wq, 64], FP, tag="wt",
                                    name=f"wt_{l}_{g}_{q}")
                    nc.vector.tensor_tensor(
                        wt[:].rearrange("p s j (h c) -> p s j h c", h=H),
                        Gq[q][:].bitcast(BF)[:, :, 0:64].rearrange(
                            "p (s j) (h c) -> p s j h c", s=nsl, h=H),
                        ext[:].unsqueeze(4).broadcast_to([128, nsl, wq, H, ch]),
                        ALU.mult)
                    if q == 0:
                        nc.vector.tensor_reduce(
                            outg[:], wt[:].rearrange("p s j f -> p s f j"),
                            axis=AX.X, op=ALU.add)
                    else:
                        wr = wpool.tile([128, nsl, 64], FP, tag="wr",
                                        name=f"wr_{l}_{g}_{q}")
                        nc.vector.tensor_reduce(
                            wr[:], wt[:].rearrange("p s j f -> p s f j"),
                            axis=AX.X, op=ALU.add)
                        nc.vector.tensor_tensor(outg[:], outg[:], wr[:], ALU.add)
                rden = wpool.tile([128, nsl, 4], FP, tag="rden", name=f"rd_{l}_{g}")
                nc.vector.reciprocal(rden[:, :, 0:H], den[:, :, 0:H])
                nc.vector.tensor_tensor(
                    outg[:].rearrange("p s (h c) -> p s h c", h=H),
                    outg[:].rearrange("p s (h c) -> p s h c", h=H),
                    rden[:, :, 0:H].unsqueeze(3).broadcast_to([128, nsl, H, ch]),
                    ALU.mult)
                # bias + outer leaky for the whole group
                nc.vector.tensor_tensor(
                    outg[:], outg[:],
                    bias_sb[:, l, :].unsqueeze(1).broadcast_to([128, nsl, 64]),
                    ALU.add)
                nc.vector.scalar_tensor_tensor(outg[:], outg[:], NEG_OUT, outg[:],
                                               ALU.mult, ALU.max)
                for si in range(nsl):
                    s = lo + si
                    if l < NL - 1:
                        pt = ppool2.tile([64, 128], FP, tag="tp", name=f"tp_{l}_{s}")
                        nc.tensor.transpose(pt[:], outg[:, si, :], ident[:])
                        xtn = xpool.tile([64, 128], FP, tag="xtn", name=f"xtn_{l}_{s}")
                        nc.scalar.copy(xtn[:], pt[:])
                        matmul_to_row(l + 1, s, xtn[:])
                    else:
                        row = rpool.tile([128, 128], I16, tag="row", name=f"rowF_{s}")
                        rf = row[:].bitcast(FP)
                        nc.vector.tensor_copy(rf[:], outg[:, si, :])
                        nc.sync.dma_start(slabs[NL][s * 128:(s + 1) * 128, :], row[:])

        # ---- final AllGather (x_final fp32 rows) ----
        nc.gpsimd.collective_compute(
            "AllGather", ALU.bypass,
            ins=[slabs[NL][:].opt()],
            outs=[tables[NL][:].opt()],
            replica_groups=[list(range(NCORES))],
        )

        # ---- pooling ----
        pidx = cpool.tile([128, 8 * WPS], I16)
        nc.sync.dma_start(pidx[:], pidx_in[:])
        pmask = cpool.tile([128, WPS], FP)
        nc.sync.dma_start(pmask[:], pmask_in[:])
        pooled = cpool.tile([128, 64], FP)
        first = True
        PCH = 32
        for q in range(NQ):
            for k0 in range(0, int(PWQS[q]), PCH):
                wq = min(PCH, int(PWQS[q]) - k0)
                c0 = int(pq0[q]) + k0
                PG = gpools[q].tile([128, wq, 128], I16, tag=f"G{q}",
                                    name=f"PG_{q}_{k0}")
                nc.gpsimd.dma_gather(
                    PG[:], tables[NL][q * QROWS:(q + 1) * QROWS, :],
                    pidx[:, 8 * c0: 8 * (c0 + wq)],
                    128 * wq, 128 * wq, 128,
                    single_packet=False, queue_num=q)
                PGf = PG[:].bitcast(FP)                    # [128, wq, 64]
                pm = wpool.tile([128, wq, 64], FP, tag="pm", name=f"pm_{q}_{k0}")
                nc.vector.tensor_tensor(
                    pm[:], PGf,
                    pmask[:, c0:c0 + wq].unsqueeze(2).broadcast_to([128, wq, 64]),
                    ALU.add)
                red = wpool.tile([128, 64], FP, tag="red", name=f"red_{q}_{k0}")
                nc.vector.tensor_reduce(red[:], pm[:].rearrange("p w f -> p f w"),
                                        axis=AX.X, op=ALU.max)
                if first:
                    nc.vector.tensor_copy(pooled[:], red[:])
                    first = False
                else:
                    nc.vector.tensor_tensor(pooled[:], pooled[:], red[:], ALU.max)
        # transpose + fold the 4 subrows per graph via strided max-reduce
        ptp = ppool2.tile([64, 128], FP, tag="tp", name="pool_tp")
        nc.tensor.transpose(ptp[:], pooled[:], ident[:])
        ptps = cpool.tile([64, 128], FP)
        nc.scalar.copy(ptps[:], ptp[:])
        pooledT = cpool.tile([64, GPC], FP)
        nc.vector.tensor_reduce(
            pooledT[:], ptps[:].rearrange("p (g r) -> p g r", r=4),
            axis=AX.X, op=ALU.max)
        # FC + bias + softmax
        fcW = cpool.tile([64, 2], FP)
        nc.sync.dma_start(fcW[:], fcW_in[:])
        fcb = cpool.tile([GPC, 2], FP)
        nc.sync.dma_start(fcb[:], fcb_in[:])
        plog = ppool.tile([GPC, 2], FP, tag="mm", name="logits_mm")
        nc.tensor.matmul(plog[:], pooledT[:], fcW[:], start=True, stop=True)
        logits = cpool.tile([GPC, 2], FP)
        nc.vector.tensor_tensor(logits[:], plog[:], fcb[:], ALU.add)
        nc.sync.dma_start(logits_out[:], logits[:])
        m = cpool.tile([GPC, 1], FP)
        nc.vector.tensor_reduce(m[:], logits[:], axis=AX.X, op=ALU.max)
        z = cpool.tile([GPC, 2], FP)
        nc.vector.tensor_tensor(z[:], logits[:], m[:].broadcast_to([GPC, 2]),
                                ALU.subtract)
        ez = cpool.tile([GPC, 2], FP)
        nc.scalar.activation(ez[:], z[:], ACTF.Exp)
        den2 = cpool.tile([GPC, 1], FP)
        nc.vector.tensor_reduce(den2[:], ez[:], axis=AX.X, op=ALU.add)
        rden2 = cpool.tile([GPC, 1], FP)
        nc.vector.reciprocal(rden2[:], den2[:])
        probas = cpool.tile([GPC, 2], FP)
        nc.vector.tensor_tensor(probas[:], ez[:], rden2[:].broadcast_to([GPC, 2]),
                                ALU.mult)
        nc.sync.dma_start(probas_out[:], probas[:])
    return nc


def make_inputs(P, inp):
    """Per-core in_maps from preprocess() result P and problem inputs."""
    BLOCKS = int(P["BLOCKS"])
    SLAB = BLOCKS * 128
    x = np.asarray(inp["x"], np.float32)
    F = x.shape[1]
    wext_np = []
    for l in range(NL):
        Wl = np.asarray(inp[f"W{l+1}"], np.float32)
        As = expand_a(np.asarray(inp[f"a{l+1}s"], np.float32))
        Ad = expand_a(np.asarray(inp[f"a{l+1}d"], np.float32))
        wext_np.append(np.concatenate([Wl, Wl @ As, Wl @ Ad], axis=1))
    bias_np = np.stack([np.asarray(inp[f"b{l+1}"], np.float32) for l in range(NL)])
    bias_rep = np.tile(bias_np[None], (128, 1, 1))
    fcW = np.asarray(inp["fcW"], np.float32)
    fcb = np.tile(np.asarray(inp["fcb"], np.float32)[None, :], (GPC, 1))

    # wrapped idx: per gather region (column range), stream = col-major
    WG, GSL, qg0, g0 = P["WG"], P["GSL"], P["qg0"], P["g0"]
    NGRP = len(GSL)
    regions = []
    for g in range(NGRP):
        for q in range(NQ):
            c0 = int(g0[g] + qg0[g][q])
            regions.append((c0, int(GSL[g]) * int(WG[g][q])))
    pregions = [(int(P["pq0"][q]), int(P["PWQS"][q])) for q in range(NQ)]

    def build_idx(lidx_c, regs):
        parts = []
        for c0, ncols in regs:
            stream = lidx_c[:, c0:c0 + ncols].T.reshape(1, -1)   # col-major
            parts.append(wrap_idx(stream)[0])
        return np.concatenate(parts, axis=1).astype(np.int16)

    in_maps = []
    for c in range(NCORES):
        nodes = P["node_at"][c]
        xs = np.zeros((SLAB, F), np.float32)
        valid = nodes >= 0
        xs[valid] = x[nodes[valid]]
        m = {
            "xT": np.ascontiguousarray(xs.T),
            "idx": build_idx(P["lidx"][c], regions),
            "pool_idx": build_idx(P["pool_lidx"][c], pregions),
            "pool_maskneg": P["pool_maskneg"][c].astype(np.float32),
            "bias": bias_rep, "fcW": fcW, "fcb": fcb,
        }
        for l in range(NL):
            m[f"wext{l}"] = wext_np[l]
        in_maps.append(m)
    return in_maps


def _run(inputs, trace=False):
    inp = {k: np.asarray(v) for k, v in inputs.items()}
    P = preprocess(inp['edge_index'], inp['batch'], N=100000, BLOCKS=98,
                   NGRAPHS=256, GS=2)
    in_maps = make_inputs(P, inp)
    nc = bacc.Bacc("TRN2", num_swdge_queues=4)
    build(nc, P)
    nc.compile()
    res = run_bass_kernel_spmd(nc, in_maps, list(range(NCORES)), trace=trace)
    global LAST_RES
    LAST_RES = res
    if trace and res.instructions_and_trace:
        print(f"trace path: {res.instructions_and_trace[1]}")
    logits = np.zeros((256, 2), np.float32)
    probas = np.zeros((256, 2), np.float32)
    for c in range(NCORES):
        lg = res.results[c]["logits"]
        pb = res.results[c]["probas"]
        for r in range(GPC):
            g = P["out_graph"][c, r]
            logits[g] = lg[r]
            probas[g] = pb[r]
    return logits, probas, res.exec_time_ns


def kernel(**inputs):
    logits, probas, _ = _run(inputs, trace=False)
    return logits, probas



# revision 8
# speedup vs baseline: 1.2781x; 1.2781x over previous
"""Self-contained distributed GAT kernel for 8 TRN2 NeuronCores (Bass/Tile).

Sharding: nodes (and incident edges, grouped by destination) across the
8 cores; weights replicated; per-layer feature tables exchanged via
AllGather; segment softmax/aggregation local per destination partition
in a rectangular [dst-row x edge-slot] layout filled by indexed DMA
gathers (4 parallel SWDGE queues, one per int16-addressable quarter).

v2: self-loop edges are never gathered (their contribution is computed
from the core-local slab copy); destination rows are packed into
256-row groups by a (argmax, sorted-degree-profile) key plus a swap
refinement to minimize rectangle padding; all gather indices stay
resident in SBUF; each group's four quarter-gathers write disjoint
column ranges of one shared G tile so the alpha-weighting runs as one
big multiply; denominators ride along the weighted reduce as 4 extra
channels ("ones trick" via the exp values themselves).

kernel(**inputs) takes FULL inputs, returns (logits, probas) float32.
"""
import sys
import numpy as np

for _p in ('/opt/trn_rl_repo', '/root/.axon_site/_ro/trn_rl_repo'):
    if _p not in sys.path:
        sys.path.append(_p)

import concourse.bacc as bacc
from concourse import mybir, masks
from concourse.tile import TileContext
from concourse.bass_utils import run_bass_kernel_spmd
from contextlib import ExitStack

NCORES = 8
NQ = 4
GS = 2


def _refine_bins(dq, ids, iters=24, nsamp=300000, seed=0):
    """Swap refinement: minimize sum over bins of per-quarter maxes.
    dq [S, R, 4] int32, ids [S, R] int64 (-1 = phantom, pinned)."""
    S, R, _ = dq.shape
    rng = np.random.default_rng(seed)
    for it in range(iters):
        top = dq.max(axis=1)
        top2 = np.partition(dq, -2, axis=1)[:, -2, :]
        s1 = rng.integers(0, S, nsamp); r1 = rng.integers(0, R, nsamp)
        s2 = rng.integers(0, S, nsamp); r2 = rng.integers(0, R, nsamp)
        ok = (ids[s1, r1] >= 0) & (ids[s2, r2] >= 0) & (s1 != s2)
        d1 = dq[s1, r1]; d2 = dq[s2, r2]
        mx_wo1 = np.where(d1 == top[s1], top2[s1], top[s1])
        mx_wo2 = np.where(d2 == top[s2], top2[s2], top[s2])
        delta = (np.maximum(mx_wo1, d2).sum(1) + np.maximum(mx_wo2, d1).sum(1)
                 - top[s1].sum(1) - top[s2].sum(1))
        good = np.where(ok & (delta < 0))[0]
        used = np.zeros(S, bool)
        cnt = 0
        for gi in good[np.argsort(delta[good], kind='stable')]:
            a, b = s1[gi], s2[gi]
            if used[a] or used[b]:
                continue
            used[a] = used[b] = True
            ra, rb = r1[gi], r2[gi]
            ids[a, ra], ids[b, rb] = ids[b, rb], ids[a, ra]
            tmp = dq[a, ra].copy(); dq[a, ra] = dq[b, rb]; dq[b, rb] = tmp
            cnt += 1
        if cnt == 0 and it > 3:
            break


def preprocess(edge_index, batch, N=100000, BLOCKS=98, NGRAPHS=256):
    NPAD = NCORES * BLOCKS * 128
    SLAB = NPAD // NCORES
    QROWS = NPAD // NQ
    GPC = NGRAPHS // NCORES
    NGRP = BLOCKS // GS
    src = np.asarray(edge_index[0], dtype=np.int64)
    dst = np.asarray(edge_index[1], dtype=np.int64)
    batch = np.asarray(batch, dtype=np.int64)
    E = src.shape[0]                      # NO self loops (handled locally)

    deg_tot = np.bincount(dst, minlength=N) + 1

    # ---- pass 1: cores by total degree (snake), quarters = core//2 ----
    order = np.argsort(-deg_tot, kind='stable')
    node_core = np.empty(N, np.int64)
    blk = np.arange(N) // 128
    s_, j_ = np.divmod(blk, NCORES)
    node_core[order] = np.where(s_ % 2 == 0, j_, NCORES - 1 - j_)
    node_quarter = node_core // 2
    degq = np.zeros((N, NQ), np.int64)
    np.add.at(degq, (dst, node_quarter[src]), 1)

    # ---- pass 2: per quarter-pair, pack into 256-row bins ----
    M64 = 64
    srt = np.sort(degq, axis=1)[:, ::-1]
    key = ((((degq.argmax(1) * M64 + srt[:, 0]) * M64 + srt[:, 1]) * M64
            + srt[:, 2]) * M64 + srt[:, 3])
    cap = 2 * BLOCKS * 128 - 2
    plists = []
    for p in range(NQ):
        nodes = np.where(node_quarter == p)[0]
        plists.append(list(nodes[np.argsort(-key[nodes], kind='stable')]))
    for p in range(NQ):                   # spill overflow to emptiest pair
        while len(plists[p]) > cap:
            tgt = min(range(NQ), key=lambda i: len(plists[i]))
            assert len(plists[tgt]) < cap
            plists[tgt].append(plists[p].pop())

    NBINS = 2 * NGRP                       # bins of 256 rows per pair
    tpos = np.empty(N, np.int64)
    Wcore = np.zeros((NCORES, NGRP, NQ), np.int64)
    core_groups = {}
    for p in range(NQ):
        arr = np.array(plists[p], dtype=np.int64)
        npad_ = NBINS * 256 - len(arr)
        assert npad_ >= 2
        ids = np.concatenate([arr, -np.ones(npad_, np.int64)]).reshape(NBINS, 256)
        dq = np.where(ids[:, :, None] >= 0, degq[np.maximum(ids, 0)], 0).astype(np.int32)
        _refine_bins(dq, ids)
        # move all phantom (-1) rows into the lightest bin's tail; that bin is
        # pinned to the even core's LAST group so the poison row lands at
        # tpos = even_core*SLAB + SLAB-1
        w = dq.max(axis=1).sum(axis=1)
        ph = np.argwhere(ids < 0)
        light = int(np.argmin(w))
        tail = 255
        for (bs, br) in ph:
            if bs == light:
                continue
            while ids[light, tail] < 0:
                tail -= 1
            ids[bs, br], ids[light, tail] = ids[light, tail], ids[bs, br]
            tmp = dq[bs, br].copy(); dq[bs, br] = dq[light, tail]; dq[light, tail] = tmp
            tail -= 1
        wb = dq.max(axis=1)
        walign = ((wb.argmax(1) * 64 + wb.max(1)) * 64 + wb.sum(1))
        phbin = light
        ordb = [b for b in np.argsort(-walign, kind='stable') if b != phbin]
        # snake over the 97 non-phantom bins: odd core first (gets 49), even 48
        ce, co = 2 * p, 2 * p + 1
        placed = []
        gidx = {ce: 0, co: 0}
        for i, b in enumerate(ordb):
            c = co if (i % 4) in (0, 3) else ce
            placed.append((c, gidx[c], b))
            gidx[c] += 1
        placed.append((ce, gidx[ce], phbin))
        gidx[ce] += 1
        assert gidx[ce] == NGRP and gidx[co] == NGRP, (gidx, NGRP)
        assert placed[-1] == (ce, NGRP - 1, phbin)
        for c, g, b in placed:
            core_groups[(c, g)] = (p, b)
            Wcore[c, g] = dq[b].max(axis=0)
            rows = ids[b]
            # ensure phantom rows (if any) sit at the very end of the bin
            if (rows < 0).any():
                rows = np.concatenate([rows[rows >= 0], rows[rows < 0]])
            val = rows >= 0
            pos = np.arange(256)
            slot = 2 * g + pos // 128
            rr = pos % 128
            tpos[rows[val]] = c * SLAB + slot[val] * 128 + rr[val]

    # program-uniform widths across cores
    WG = np.maximum(Wcore.max(axis=0), 1)          # [NGRP, NQ]
    # widen to actual needs (safety: recompute from real geometry below)

    d_t = tpos[dst]
    d_core = d_t // SLAB
    d_rem = d_t - d_core * SLAB
    d_slot = d_rem // 128
    d_row = d_rem - d_slot * 128
    d_grp = d_slot // GS
    d_sloc = d_slot - d_grp * GS
    s_t = tpos[src]
    s_q = s_t // QROWS
    s_local = (s_t - s_q * QROWS).astype(np.int16)

    cnt = np.zeros((NCORES, BLOCKS, 128, NQ), np.int64)
    np.add.at(cnt, (d_core, d_slot, d_row, s_q), 1)
    need = cnt.reshape(NCORES, NGRP, GS, 128, NQ).max(axis=(0, 2, 3))
    WG = np.maximum(WG, need)                       # [NGRP, NQ]

    # group column geometry: per group, cols = [q][slot-in-group][W]
    qg0 = np.zeros((NGRP, NQ + 1), np.int64)
    for g in range(NGRP):
        qg0[g, 1:] = np.cumsum(WG[g] * GS)
    GW = qg0[:, -1]
    g0 = np.concatenate([[0], np.cumsum(GW)])
    WTOT = int(g0[-1])

    # rank within (core, slot, row, quarter)
    kk = ((d_core * BLOCKS + d_slot) * 128 + d_row) * NQ + s_q
    eorder = np.argsort(kk, kind='stable')
    ks = kk[eorder]
    first = np.concatenate([[True], ks[1:] != ks[:-1]])
    runstart = np.maximum.accumulate(np.where(first, np.arange(E), 0))
    rank = np.arange(E) - runstart

    lidx = np.full((NCORES, 128, WTOT), SLAB - 1, np.int16)  # poison row
    eo = eorder
    col = (g0[d_grp[eo]] + qg0[d_grp[eo], s_q[eo]]
           + d_sloc[eo] * WG[d_grp[eo], s_q[eo]] + rank)
    assert (rank < WG[d_grp[eo], s_q[eo]]).all()
    lidx[d_core[eo], d_row[eo], col] = s_local[eo]

    node_at = np.full((NCORES, SLAB), -1, np.int64)
    lp = tpos - (tpos // SLAB) * SLAB
    node_at[tpos // SLAB, lp] = np.arange(N)
    for q in range(NQ):
        assert node_at[2 * q, SLAB - 1] == -1, "even-core phantom poison row"

    # ---- pooling: graph g -> (core, grow); nodes on partitions
    # 4*grow + subrow (subrow = within-(graph,quarter) rank % 4) ----
    gsize = np.bincount(batch, minlength=NGRAPHS)
    gorder = np.argsort(-gsize, kind='stable')
    pool_core = np.empty(NGRAPHS, np.int64)
    pool_row = np.empty(NGRAPHS, np.int64)
    for i, g in enumerate(gorder):
        r, j = divmod(i, NCORES)
        pool_core[g] = j if r % 2 == 0 else NCORES - 1 - j
        pool_row[g] = r

    keyp = (pool_core[batch] * GPC + pool_row[batch]) * NQ + (tpos // QROWS)
    porder = np.argsort(keyp, kind='stable')
    kp = keyp[porder]
    firstp = np.concatenate([[True], kp[1:] != kp[:-1]])
    runstart = np.maximum.accumulate(np.where(firstp, np.arange(N), 0))
    rankp = np.arange(N) - runstart
    subrow = rankp % 4
    jcol = rankp // 4
    pq = np.zeros((NCORES, GPC, NQ), np.int64)
    np.add.at(pq, (pool_core[batch], pool_row[batch], tpos // QROWS), 1)
    PWQS = np.maximum((pq + 3) // 4, 1).max(axis=(0, 1))   # [NQ]
    pq0 = np.concatenate([[0], np.cumsum(PWQS)])
    WPS = int(pq0[-1])

    pool_lidx = np.zeros((NCORES, 128, WPS), np.int16)
    pool_pad = np.ones((NCORES, 128, WPS), bool)
    pc = pool_core[batch][porder]
    pr = pool_row[batch][porder]
    ppart = pr * 4 + subrow
    pcol = pq0[(tpos // QROWS)[porder]] + jcol
    pool_lidx[pc, ppart, pcol] = (tpos - (tpos // QROWS) * QROWS)[porder].astype(np.int16)
    pool_pad[pc, ppart, pcol] = False
    pool_maskneg = np.where(pool_pad, np.float32(-1e30), np.float32(0.0))
    out_graph = np.empty((NCORES, GPC), np.int64)
    out_graph[pool_core, pool_row] = np.arange(NGRAPHS)

    pad_factor = WTOT * 128 * NCORES / max(E, 1)
    return dict(
        tpos=tpos, node_at=node_at, BLOCKS=BLOCKS,
        lidx=lidx, WG=WG, qg0=qg0, g0=g0, WTOT=WTOT,
        pool_lidx=pool_lidx, pool_maskneg=pool_maskneg, PWQS=PWQS, pq0=pq0,
        WPS=WPS, out_graph=out_graph, pad_factor=pad_factor,
    )


def wrap_idx(vals):
    """[..., n] int16, n % 16 == 0: idx i -> [i%16, i//16], replicated x8 to
    128 partitions -> [..., 128, n/16]."""
    sh = vals.shape[:-1]
    n = vals.shape[-1]
    assert n % 16 == 0
    w = vals.reshape(*sh, n // 16, 16)
    w = np.swapaxes(w, -1, -2)
    w = np.broadcast_to(w[..., None, :, :], (*sh, 8, 16, n // 16))
    return w.reshape(*sh, 128, n // 16).copy()


def expand_a(a):
    heads, ch = a.shape
    A = np.zeros((heads * ch, heads), np.float32)
    for h in range(heads):
        A[h * ch:(h + 1) * ch, h] = a[h]
    return A


FP = mybir.dt.float32
BF = mybir.dt.bfloat16
I16 = mybir.dt.int16
ALU = mybir.AluOpType
ACTF = mybir.ActivationFunctionType
AX = mybir.AxisListType

NL = 4
GPC = 32
NEG = 0.2
NEG_OUT = 0.01
HEADS = (4, 4, 4, 1)


def build(nc, geom):
    BLOCKS = int(geom["BLOCKS"])
    NP_ = NCORES * BLOCKS * 128
    SLAB = NP_ // NCORES
    QROWS = NP_ // NQ
    WG = np.asarray(geom["WG"])            # [NGRP, NQ]
    qg0 = np.asarray(geom["qg0"])          # [NGRP, NQ+1]
    g0 = np.asarray(geom["g0"])            # [NGRP+1]
    WTOT = int(geom["WTOT"])
    PWQS = np.asarray(geom["PWQS"])        # [NQ]
    pq0 = np.asarray(geom["pq0"])          # [NQ+1]
    WPS = int(geom["WPS"])
    NGRP = BLOCKS // GS

    # ---------------- I/O ----------------
    xT = nc.declare_dram_parameter("xT", [128, SLAB], FP, isOutput=False)
    idx_in = nc.declare_dram_parameter("idx", [128, 8 * WTOT], I16, isOutput=False)
    pidx_in = nc.declare_dram_parameter("pool_idx", [128, 8 * WPS], I16, isOutput=False)
    pmask_in = nc.declare_dram_parameter("pool_maskneg", [128, WPS], FP, isOutput=False)
    wext_in = [
        nc.declare_dram_parameter(f"wext{l}", [128 if l == 0 else 64, 64 + 2 * HEADS[l]],
                                  FP, isOutput=False)
        for l in range(NL)
    ]
    bias_in = nc.declare_dram_parameter("bias", [128, NL, 64], FP, isOutput=False)
    fcW_in = nc.declare_dram_parameter("fcW", [64, 2], FP, isOutput=False)
    fcb_in = nc.declare_dram_parameter("fcb", [GPC, 2], FP, isOutput=False)
    logits_out = nc.declare_dram_parameter("logits", [GPC, 2], FP, isOutput=True)
    probas_out = nc.declare_dram_parameter("probas", [GPC, 2], FP, isOutput=True)

    with TileContext(nc) as tc, ExitStack() as ex:
        dram = ex.enter_context(tc.tile_pool(name="dram", bufs=1, space="DRAM"))
        tables = [dram.tile([NP_, 128], I16, addr_space="Shared", name=f"table{l}")
                  for l in range(NL + 1)]
        slabs = [dram.tile([SLAB, 128], I16, name=f"slab{l}") for l in range(NL + 1)]

        cpool = ex.enter_context(tc.tile_pool(name="const", bufs=1))
        gpool = ex.enter_context(tc.tile_pool(name="gath", bufs=2))
        epool = ex.enter_context(tc.tile_pool(name="esc", bufs=3))
        wtpool = ex.enter_context(tc.tile_pool(name="wt", bufs=2))
        mpool = ex.enter_context(tc.tile_pool(name="pmx", bufs=2))
        apool = ex.enter_context(tc.tile_pool(name="acc", bufs=2))
        opool = ex.enter_context(tc.tile_pool(name="outg", bufs=2))
        xpool = ex.enter_context(tc.tile_pool(name="xin", bufs=3))
        ppool = ex.enter_context(tc.tile_pool(name="psum", bufs=4, space="PSUM"))
        ppool2 = ex.enter_context(tc.tile_pool(name="psum2", bufs=2, space="PSUM"))

        # ---- constants / resident tiles ----
        wext = []
        for l in range(NL):
            t = cpool.tile([128 if l == 0 else 64, 64 + 2 * HEADS[l]], FP,
                           name=f"wext_sb{l}")
            nc.sync.dma_start(t[:], wext_in[l][:])
            wext.append(t)
        bias_sb = cpool.tile([128, NL, 64], FP)
        nc.sync.dma_start(bias_sb[:], bias_in[:])
        ident = cpool.tile([128, 128], FP)
        masks.make_identity(nc, ident[:])
        poison_t = cpool.tile([1, 4], FP)
        nc.vector.memset(poison_t[:], -120.0)
        idx_sb = cpool.tile([128, 8 * WTOT], I16)
        nc.sync.dma_start(idx_sb[:], idx_in[:])
        pidx = cpool.tile([128, 8 * WPS], I16)
        nc.sync.dma_start(pidx[:], pidx_in[:])
        pmask = cpool.tile([128, WPS], FP)
        nc.sync.dma_start(pmask[:], pmask_in[:])
        # resident local slab (this core's rows, layer-current)
        slab_sb = cpool.tile([128, BLOCKS, 128], I16)
        slab_f = slab_sb[:].bitcast(FP)           # [128, BLOCKS, 64]
        eself = [cpool.tile([128, BLOCKS, 4], FP, name=f"eself{i}") for i in range(2)]

        def matmul_to_row(l, s, lhsT_ap):
            """h_ext = lhsT.T @ wext[l] -> slab_sb[:, s, :] (bf16 h + fp32
            scores) -> DRAM slab row block."""
            H = HEADS[l]
            pm = ppool.tile([128, 64 + 2 * H], FP, tag="mm", name=f"mm_{l}_{s}")
            nc.tensor.matmul(pm[:], lhsT_ap, wext[l][:], start=True, stop=True)
            nc.scalar.copy(slab_sb[:].bitcast(BF)[:, s, 0:64], pm[:, 0:64])
            nc.vector.tensor_copy(slab_f[:, s, 32:32 + 2 * H], pm[:, 64:64 + 2 * H])
            nc.vector.memset(slab_f[:, s, 32 + 2 * H:64], 0.0)
            nc.sync.dma_start(slabs[l][s * 128:(s + 1) * 128, :], slab_sb[:, s, :])
            if s == BLOCKS - 1:
                # poison row: phantom last slab row's s_src <- -120 so padded
                # gather slots vanish in the softmax
                nc.sync.dma_start(slabs[l][SLAB - 1:SLAB, 64:72],
                                  poison_t[:].bitcast(I16))

        # ---- layer-0 matmul phase ----
        for s in range(BLOCKS):
            xt = xpool.tile([128, 128], FP, tag="xt", name=f"xt0_{s}")
            nc.sync.dma_start(xt[:], xT[:, s * 128:(s + 1) * 128])
            matmul_to_row(0, s, xt[:])

        # ---- layers ----
        for l in range(NL):
            nc.gpsimd.collective_compute(
                "AllGather", ALU.bypass,
                ins=[slabs[l][:].opt()],
                outs=[tables[l][:].opt()],
                replica_groups=[list(range(NCORES))],
            )
            H = HEADS[l]
            ch = 64 // H
            CH = 64 + H                       # wt channels: 64 feat + H den
            # self attention scores for this layer (from resident slab)
            es = eself[l % 2]
            nc.vector.tensor_tensor(es[:, :, 0:H], slab_f[:, :, 32:32 + H],
                                    slab_f[:, :, 32 + H:32 + 2 * H], ALU.add)
            nc.vector.scalar_tensor_tensor(es[:, :, 0:H], es[:, :, 0:H], NEG,
                                           es[:, :, 0:H], ALU.mult, ALU.max)
            nc.scalar.activation(es[:, :, 0:H], es[:, :, 0:H], ACTF.Exp)

            for g in range(NGRP):
                GWg = int(g0[g + 1] - g0[g])
                G = gpool.tile([128, GWg, 128], I16, tag="G", name=f"G_{l}_{g}")
                for q in range(NQ):
                    wq = int(WG[g][q])
                    ncols = GS * wq
                    r0 = int(qg0[g][q])
                    nc.gpsimd.dma_gather(
                        G[:, r0:r0 + ncols, :],
                        tables[l][q * QROWS:(q + 1) * QROWS, :],
                        idx_sb[:, 8 * int(g0[g] + r0): 8 * int(g0[g] + r0 + ncols)],
                        128 * ncols, 128 * ncols, 128,
                        single_packet=False, queue_num=q)
                Gf = G[:].bitcast(FP)                  # [128, GWg, 64]
                Gb = G[:].bitcast(BF)                  # [128, GWg, 128]
                wt = wtpool.tile([128, GWg, CH], BF, tag="wt", name=f"wt_{l}_{g}")
                for q in range(NQ):
                    wq = int(WG[g][q])
                    r0 = int(qg0[g][q])
                    ssrc = Gf[:, r0:r0 + GS * wq, 32:32 + H].rearrange(
                        "p (s j) h -> p s j h", s=GS)
                    e = epool.tile([128, GS, wq, H], FP, tag="e",
                                   name=f"e_{l}_{g}_{q}")
                    nc.vector.tensor_tensor(
                        e[:], ssrc,
                        slab_f[:, GS * g:GS * (g + 1), 32 + H:32 + 2 * H]
                        .unsqueeze(2).broadcast_to([128, GS, wq, H]),
                        ALU.add)
                    nc.vector.scalar_tensor_tensor(e[:], e[:], NEG, e[:],
                                                   ALU.mult, ALU.max)
                    nc.scalar.activation(
                        wt[:, r0:r0 + GS * wq, 64:64 + H].rearrange(
                            "p (s j) h -> p s j h", s=GS),
                        e[:], ACTF.Exp)
                # one big alpha-weighting multiply over all quarters
                nc.vector.tensor_tensor(
                    wt[:, :, 0:64].rearrange("p c (h k) -> p c h k", h=H),
                    Gb[:, :, 0:64].rearrange("p c (h k) -> p c h k", h=H),
                    wt[:, :, 64:64 + H].unsqueeze(3).broadcast_to([128, GWg, H, ch]),
                    ALU.mult)
                # accumulator: [128, GS, CH, NQ+1]; slot NQ = self term
                acc = apool.tile([128, GS, CH, NQ + 1], FP, tag="acc",
                                 name=f"acc_{l}_{g}")
                nc.vector.tensor_tensor(
                    acc[:, :, 0:64, NQ].rearrange("p s (h k) -> p s h k", h=H),
                    slab_sb[:].bitcast(BF)[:, GS * g:GS * (g + 1), 0:64].rearrange(
                        "p s (h k) -> p s h k", h=H),
                    es[:, GS * g:GS * (g + 1), 0:H].unsqueeze(3)
                    .broadcast_to([128, GS, H, ch]),
                    ALU.mult)
                nc.vector.tensor_copy(acc[:, :, 64:64 + H, NQ],
                                      es[:, GS * g:GS * (g + 1), 0:H])
                for q in range(NQ):
                    wq = int(WG[g][q])
                    r0 = int(qg0[g][q])
                    nc.vector.tensor_reduce(
                        acc[:, :, :, q],
                        wt[:, r0:r0 + GS * wq, :].rearrange(
                            "p (s j) c -> p s c j", s=GS),
                        axis=AX.X, op=ALU.add)
                tot = apool.tile([128, GS, CH], FP, tag="tot", name=f"tot_{l}_{g}")
                nc.vector.tensor_reduce(tot[:], acc[:], axis=AX.X, op=ALU.add)
                rden = apool.tile([128, GS, H], FP, tag="rd", name=f"rd_{l}_{g}")
                nc.vector.reciprocal(rden[:], tot[:, :, 64:64 + H])
                outg = opool.tile([128, GS, 64], FP, tag="outg", name=f"og_{l}_{g}")
                nc.vector.tensor_tensor(
                    outg[:].rearrange("p s (h k) -> p s h k", h=H),
                    tot[:, :, 0:64].rearrange("p s (h k) -> p s h k", h=H),
                    rden[:].unsqueeze(3).broadcast_to([128, GS, H, ch]),
                    ALU.mult)
                nc.vector.tensor_tensor(
                    outg[:], outg[:],
                    bias_sb[:, l, :].unsqueeze(1).broadcast_to([128, GS, 64]),
                    ALU.add)
                nc.vector.scalar_tensor_tensor(outg[:], outg[:], NEG_OUT, outg[:],
                                               ALU.mult, ALU.max)
                for si in range(GS):
                    s = GS * g + si
                    if l < NL - 1:
                        pt = ppool2.tile([64, 128], FP, tag="tp", name=f"tp_{l}_{s}")
                        nc.tensor.transpose(pt[:], outg[:, si, :], ident[:])
                        xtn = xpool.tile([64, 128], FP, tag="xtn", name=f"xtn_{l}_{s}")
                        nc.scalar.copy(xtn[:], pt[:])
                        matmul_to_row(l + 1, s, xtn[:])
                    else:
                        rowf = slab_f[:, s, :]
                        nc.vector.tensor_copy(rowf, outg[:, si, :])
                        nc.sync.dma_start(slabs[NL][s * 128:(s + 1) * 128, :],
                                          slab_sb[:, s, :])

        # ---- final AllGather (x_final fp32 rows) ----
        nc.gpsimd.collective_compute(
            "AllGather", ALU.bypass,
            ins=[slabs[NL][:].opt()],
            outs=[tables[NL][:].opt()],
            replica_groups=[list(range(NCORES))],
        )

        # ---- pooling ----
        pooled = cpool.tile([128, 64], FP)
        first = True
        PCH = 16
        for q in range(NQ):
            for k0 in range(0, int(PWQS[q]), PCH):
                wq = min(PCH, int(PWQS[q]) - k0)
                c0 = int(pq0[q]) + k0
                PG = gpool.tile([128, wq, 128], I16, tag="G",
                                name=f"PG_{q}_{k0}")
                nc.gpsimd.dma_gather(
                    PG[:], tables[NL][q * QROWS:(q + 1) * QROWS, :],
                    pidx[:, 8 * c0: 8 * (c0 + wq)],
                    128 * wq, 128 * wq, 128,
                    single_packet=False, queue_num=q)
                PGf = PG[:].bitcast(FP)                    # [128, wq, 64]
                pm = mpool.tile([128, wq, 64], FP, tag="pm", name=f"pm_{q}_{k0}")
                nc.vector.tensor_tensor(
                    pm[:], PGf,
                    pmask[:, c0:c0 + wq].unsqueeze(2).broadcast_to([128, wq, 64]),
                    ALU.add)
                red = mpool.tile([128, 64], FP, tag="red", name=f"red_{q}_{k0}")
                nc.vector.tensor_reduce(red[:], pm[:].rearrange("p w f -> p f w"),
                                        axis=AX.X, op=ALU.max)
                if first:
                    nc.vector.tensor_copy(pooled[:], red[:])
                    first = False
                else:
                    nc.vector.tensor_tensor(pooled[:], pooled[:], red[:], ALU.max)
        # transpose + fold the 4 subrows per graph via strided max-reduce
        ptp = ppool2.tile([64, 128], FP, tag="tp", name="pool_tp")
        nc.tensor.transpose(ptp[:], pooled[:], ident[:])
        ptps = cpool.tile([64, 128], FP)
        nc.scalar.copy(ptps[:], ptp[:])
        pooledT = cpool.tile([64, GPC], FP)
        nc.vector.tensor_reduce(
            pooledT[:], ptps[:].rearrange("p (g r) -> p g r", r=4),
            axis=AX.X, op=ALU.max)
        # FC + bias + softmax
        fcW = cpool.tile([64, 2], FP)
        nc.sync.dma_start(fcW[:], fcW_in[:])
        fcb = cpool.tile([GPC, 2], FP)
        nc.sync.dma_start(fcb[:], fcb_in[:])
        plog = ppool.tile([GPC, 2], FP, tag="mm", name="logits_mm")
        nc.tensor.matmul(plog[:], pooledT[:], fcW[:], start=True, stop=True)
        logits = cpool.tile([GPC, 2], FP)
        nc.vector.tensor_tensor(logits[:], plog[:], fcb[:], ALU.add)
        nc.sync.dma_start(logits_out[:], logits[:])
        m = cpool.tile([GPC, 1], FP)
        nc.vector.tensor_reduce(m[:], logits[:], axis=AX.X, op=ALU.max)
        z = cpool.tile([GPC, 2], FP)
        nc.vector.tensor_tensor(z[:], logits[:], m[:].broadcast_to([GPC, 2]),
                                ALU.subtract)
        ez = cpool.tile([GPC, 2], FP)
        nc.scalar.activation(ez[:], z[:], ACTF.Exp)
        den2 = cpool.tile([GPC, 1], FP)
        nc.vector.tensor_reduce(den2[:], ez[:], axis=AX.X, op=ALU.add)
        rden2 = cpool.tile([GPC, 1], FP)
        nc.vector.reciprocal(rden2[:], den2[:])
        probas = cpool.tile([GPC, 2], FP)
        nc.vector.tensor_tensor(probas[:], ez[:], rden2[:].broadcast_to([GPC, 2]),
                                ALU.mult)
        nc.sync.dma_start(probas_out[:], probas[:])
    return nc


def make_inputs(P, inp):
    """Per-core in_maps from preprocess() result P and problem inputs."""
    BLOCKS = int(P["BLOCKS"])
    SLAB = BLOCKS * 128
    x = np.asarray(inp["x"], np.float32)
    F = x.shape[1]
    wext_np = []
    for l in range(NL):
        Wl = np.asarray(inp[f"W{l+1}"], np.float32)
        As = expand_a(np.asarray(inp[f"a{l+1}s"], np.float32))
        Ad = expand_a(np.asarray(inp[f"a{l+1}d"], np.float32))
        wext_np.append(np.concatenate([Wl, Wl @ As, Wl @ Ad], axis=1))
    bias_np = np.stack([np.asarray(inp[f"b{l+1}"], np.float32) for l in range(NL)])
    bias_rep = np.tile(bias_np[None], (128, 1, 1))
    fcW = np.asarray(inp["fcW"], np.float32)
    fcb = np.tile(np.asarray(inp["fcb"], np.float32)[None, :], (GPC, 1))

    WG, qg0, g0 = P["WG"], P["qg0"], P["g0"]
    NGRP = BLOCKS // GS
    regions = []
    for g in range(NGRP):
        for q in range(NQ):
            c0 = int(g0[g] + qg0[g][q])
            regions.append((c0, GS * int(WG[g][q])))
    pregions = [(int(P["pq0"][q]), int(P["PWQS"][q])) for q in range(NQ)]

    def build_idx(lidx_c, regs):
        parts = []
        for c0, ncols in regs:
            stream = lidx_c[:, c0:c0 + ncols].T.reshape(1, -1)   # col-major
            parts.append(wrap_idx(stream)[0])
        return np.concatenate(parts, axis=1).astype(np.int16)

    in_maps = []
    for c in range(NCORES):
        nodes = P["node_at"][c]
        xs = np.zeros((SLAB, F), np.float32)
        valid = nodes >= 0
        xs[valid] = x[nodes[valid]]
        m = {
            "xT": np.ascontiguousarray(xs.T),
            "idx": build_idx(P["lidx"][c], regions),
            "pool_idx": build_idx(P["pool_lidx"][c], pregions),
            "pool_maskneg": P["pool_maskneg"][c].astype(np.float32),
            "bias": bias_rep, "fcW": fcW, "fcb": fcb,
        }
        for l in range(NL):
            m[f"wext{l}"] = wext_np[l]
        in_maps.append(m)
    return in_maps


def _run(inputs, trace=False):
    inp = {k: np.asarray(v) for k, v in inputs.items()}
    P = preprocess(inp['edge_index'], inp['batch'], N=100000, BLOCKS=98,
                   NGRAPHS=256)
    print(f"pad_factor: {P['pad_factor']:.3f}, WTOT={P['WTOT']}")
    in_maps = make_inputs(P, inp)
    nc = bacc.Bacc("TRN2", num_swdge_queues=4)
    build(nc, P)
    nc.compile()
    res = run_bass_kernel_spmd(nc, in_maps, list(range(NCORES)), trace=trace)
    global LAST_RES
    LAST_RES = res
    if trace and res.instructions_and_trace:
        print(f"trace path: {res.instructions_and_trace[1]}")
    logits = np.zeros((256, 2), np.float32)
    probas = np.zeros((256, 2), np.float32)
    for c in range(NCORES):
        lg = res.results[c]["logits"]
        pb = res.results[c]["probas"]
        for r in range(GPC):
            g = P["out_graph"][c, r]
            logits[g] = lg[r]
            probas[g] = pb[r]
    return logits, probas, res.exec_time_ns


def kernel(**inputs):
    logits, probas, _ = _run(inputs, trace=False)
    return logits, probas


# revision 14
# speedup vs baseline: 1.4402x; 1.1268x over previous
"""Self-contained distributed GAT kernel for 8 TRN2 NeuronCores (Bass/Tile).

Sharding: nodes (and incident edges, grouped by destination) across the
8 cores; weights replicated; per-layer feature tables exchanged via
AllGather; segment softmax/aggregation local per destination partition
in a rectangular [dst-row x edge-slot] layout filled by indexed DMA
gathers (4 parallel SWDGE queues, one per int16-addressable quarter).

v2: self-loop edges are never gathered (their contribution is computed
from the core-local slab copy); destination rows are packed into
256-row groups by a (argmax, sorted-degree-profile) key plus a swap
refinement to minimize rectangle padding; all gather indices stay
resident in SBUF; each group's four quarter-gathers write disjoint
column ranges of one shared G tile so the alpha-weighting runs as one
big multiply; denominators ride along the weighted reduce as 4 extra
channels ("ones trick" via the exp values themselves).

kernel(**inputs) takes FULL inputs, returns (logits, probas) float32.
"""
import sys
import numpy as np

for _p in ('/opt/trn_rl_repo', '/root/.axon_site/_ro/trn_rl_repo'):
    if _p not in sys.path:
        sys.path.append(_p)

import concourse.bacc as bacc
from concourse import mybir, masks
from concourse.tile import TileContext
from concourse.bass_utils import run_bass_kernel_spmd
from contextlib import ExitStack

NCORES = 8
NQ = 4
GS = 2


def _refine_bins(dq, ids, iters=24, nsamp=300000, seed=0):
    """Swap refinement: minimize sum over bins of per-quarter maxes.
    dq [S, R, 4] int32, ids [S, R] int64 (-1 = phantom, pinned)."""
    S, R, _ = dq.shape
    rng = np.random.default_rng(seed)
    for it in range(iters):
        top = dq.max(axis=1)
        top2 = np.partition(dq, -2, axis=1)[:, -2, :]
        s1 = rng.integers(0, S, nsamp); r1 = rng.integers(0, R, nsamp)
        s2 = rng.integers(0, S, nsamp); r2 = rng.integers(0, R, nsamp)
        ok = (ids[s1, r1] >= 0) & (ids[s2, r2] >= 0) & (s1 != s2)
        d1 = dq[s1, r1]; d2 = dq[s2, r2]
        mx_wo1 = np.where(d1 == top[s1], top2[s1], top[s1])
        mx_wo2 = np.where(d2 == top[s2], top2[s2], top[s2])
        delta = (np.maximum(mx_wo1, d2).sum(1) + np.maximum(mx_wo2, d1).sum(1)
                 - top[s1].sum(1) - top[s2].sum(1))
        good = np.where(ok & (delta < 0))[0]
        used = np.zeros(S, bool)
        cnt = 0
        for gi in good[np.argsort(delta[good], kind='stable')]:
            a, b = s1[gi], s2[gi]
            if used[a] or used[b]:
                continue
            used[a] = used[b] = True
            ra, rb = r1[gi], r2[gi]
            ids[a, ra], ids[b, rb] = ids[b, rb], ids[a, ra]
            tmp = dq[a, ra].copy(); dq[a, ra] = dq[b, rb]; dq[b, rb] = tmp
            cnt += 1
        if cnt == 0 and it > 3:
            break


def preprocess(edge_index, batch, N=100000, BLOCKS=98, NGRAPHS=256):
    NPAD = NCORES * BLOCKS * 128
    SLAB = NPAD // NCORES
    QROWS = NPAD // NQ
    GPC = NGRAPHS // NCORES
    NGRP = BLOCKS // GS
    src = np.asarray(edge_index[0], dtype=np.int64)
    dst = np.asarray(edge_index[1], dtype=np.int64)
    batch = np.asarray(batch, dtype=np.int64)
    E = src.shape[0]                      # NO self loops (handled locally)

    deg_tot = np.bincount(dst, minlength=N) + 1

    # ---- pass 1: cores by total degree (snake), quarters = core//2 ----
    order = np.argsort(-deg_tot, kind='stable')
    node_core = np.empty(N, np.int64)
    blk = np.arange(N) // 128
    s_, j_ = np.divmod(blk, NCORES)
    node_core[order] = np.where(s_ % 2 == 0, j_, NCORES - 1 - j_)
    node_quarter = node_core // 2
    degq = np.zeros((N, NQ), np.int64)
    np.add.at(degq, (dst, node_quarter[src]), 1)

    # ---- pass 2: per quarter-pair, pack into 256-row bins ----
    M64 = 64
    srt = np.sort(degq, axis=1)[:, ::-1]
    key = ((((degq.argmax(1) * M64 + srt[:, 0]) * M64 + srt[:, 1]) * M64
            + srt[:, 2]) * M64 + srt[:, 3])
    cap = 2 * BLOCKS * 128 - 2
    plists = []
    for p in range(NQ):
        nodes = np.where(node_quarter == p)[0]
        plists.append(list(nodes[np.argsort(-key[nodes], kind='stable')]))
    for p in range(NQ):                   # spill overflow to emptiest pair
        while len(plists[p]) > cap:
            tgt = min(range(NQ), key=lambda i: len(plists[i]))
            assert len(plists[tgt]) < cap
            plists[tgt].append(plists[p].pop())

    NBINS = 2 * NGRP                       # bins of GS*128 rows per pair
    RB = GS * 128
    tpos = np.empty(N, np.int64)
    Wcore = np.zeros((NCORES, NGRP, NQ), np.int64)
    core_groups = {}
    for p in range(NQ):
        arr = np.array(plists[p], dtype=np.int64)
        npad_ = NBINS * RB - len(arr)
        assert npad_ >= 2
        ids = np.concatenate([arr, -np.ones(npad_, np.int64)]).reshape(NBINS, RB)
        dq = np.where(ids[:, :, None] >= 0, degq[np.maximum(ids, 0)], 0).astype(np.int32)
        _refine_bins(dq, ids)
        # pin one phantom-bearing bin to the even core's LAST group so the
        # poison row lands at tpos = even_core*SLAB + SLAB-1 (rows are
        # reordered real-first below, putting a phantom at bin tail)
        nph = (ids < 0).sum(axis=1)
        light = int(np.argmax(nph))
        assert nph[light] >= 1
        wb = dq.max(axis=1)
        walign = ((wb.argmax(1) * 64 + wb.max(1)) * 64 + wb.sum(1))
        phbin = light
        ordb = [b for b in np.argsort(-walign, kind='stable') if b != phbin]
        # snake over the 97 non-phantom bins: odd core first (gets 49), even 48
        ce, co = 2 * p, 2 * p + 1
        placed = []
        gidx = {ce: 0, co: 0}
        for i, b in enumerate(ordb):
            c = co if (i % 4) in (0, 3) else ce
            placed.append((c, gidx[c], b))
            gidx[c] += 1
        placed.append((ce, gidx[ce], phbin))
        gidx[ce] += 1
        assert gidx[ce] == NGRP and gidx[co] == NGRP, (gidx, NGRP)
        assert placed[-1] == (ce, NGRP - 1, phbin)
        for c, g, b in placed:
            core_groups[(c, g)] = (p, b)
            Wcore[c, g] = dq[b].max(axis=0)
            rows = ids[b]
            # ensure phantom rows (if any) sit at the very end of the bin
            if (rows < 0).any():
                rows = np.concatenate([rows[rows >= 0], rows[rows < 0]])
            val = rows >= 0
            pos = np.arange(RB)
            slot = GS * g + pos // 128
            rr = pos % 128
            tpos[rows[val]] = c * SLAB + slot[val] * 128 + rr[val]

    # program-uniform widths across cores
    WG = np.maximum(Wcore.max(axis=0), 1)          # [NGRP, NQ]
    # widen to actual needs (safety: recompute from real geometry below)

    d_t = tpos[dst]
    d_core = d_t // SLAB
    d_rem = d_t - d_core * SLAB
    d_slot = d_rem // 128
    d_row = d_rem - d_slot * 128
    d_grp = d_slot // GS
    d_sloc = d_slot - d_grp * GS
    s_t = tpos[src]
    s_q = s_t // QROWS
    s_local = (s_t - s_q * QROWS).astype(np.int16)

    cnt = np.zeros((NCORES, BLOCKS, 128, NQ), np.int64)
    np.add.at(cnt, (d_core, d_slot, d_row, s_q), 1)
    need = cnt.reshape(NCORES, NGRP, GS, 128, NQ).max(axis=(0, 2, 3))
    WG = np.maximum(WG, need)                       # [NGRP, NQ]

    # group column geometry: per group, cols = [q][slot-in-group][W]
    qg0 = np.zeros((NGRP, NQ + 1), np.int64)
    for g in range(NGRP):
        qg0[g, 1:] = np.cumsum(WG[g] * GS)
    GW = qg0[:, -1]
    g0 = np.concatenate([[0], np.cumsum(GW)])
    WTOT = int(g0[-1])

    # rank within (core, slot, row, quarter)
    kk = ((d_core * BLOCKS + d_slot) * 128 + d_row) * NQ + s_q
    eorder = np.argsort(kk, kind='stable')
    ks = kk[eorder]
    first = np.concatenate([[True], ks[1:] != ks[:-1]])
    runstart = np.maximum.accumulate(np.where(first, np.arange(E), 0))
    rank = np.arange(E) - runstart

    lidx = np.full((NCORES, 128, WTOT), SLAB - 1, np.int16)  # poison row
    eo = eorder
    col = (g0[d_grp[eo]] + qg0[d_grp[eo], s_q[eo]]
           + d_sloc[eo] * WG[d_grp[eo], s_q[eo]] + rank)
    assert (rank < WG[d_grp[eo], s_q[eo]]).all()
    lidx[d_core[eo], d_row[eo], col] = s_local[eo]

    node_at = np.full((NCORES, SLAB), -1, np.int64)
    lp = tpos - (tpos // SLAB) * SLAB
    node_at[tpos // SLAB, lp] = np.arange(N)
    for q in range(NQ):
        assert node_at[2 * q, SLAB - 1] == -1, "even-core phantom poison row"

    # ---- pooling: graph g -> (core, grow); nodes on partitions
    # 4*grow + subrow (subrow = within-(graph,quarter) rank % 4) ----
    gsize = np.bincount(batch, minlength=NGRAPHS)
    gorder = np.argsort(-gsize, kind='stable')
    pool_core = np.empty(NGRAPHS, np.int64)
    pool_row = np.empty(NGRAPHS, np.int64)
    for i, g in enumerate(gorder):
        r, j = divmod(i, NCORES)
        pool_core[g] = j if r % 2 == 0 else NCORES - 1 - j
        pool_row[g] = r

    keyp = (pool_core[batch] * GPC + pool_row[batch]) * NQ + (tpos // QROWS)
    porder = np.argsort(keyp, kind='stable')
    kp = keyp[porder]
    firstp = np.concatenate([[True], kp[1:] != kp[:-1]])
    runstart = np.maximum.accumulate(np.where(firstp, np.arange(N), 0))
    rankp = np.arange(N) - runstart
    subrow = rankp % 4
    jcol = rankp // 4
    pq = np.zeros((NCORES, GPC, NQ), np.int64)
    np.add.at(pq, (pool_core[batch], pool_row[batch], tpos // QROWS), 1)
    PWQS = np.maximum((pq + 3) // 4, 1).max(axis=(0, 1))   # [NQ]
    pq0 = np.concatenate([[0], np.cumsum(PWQS)])
    WPS = int(pq0[-1])

    pool_lidx = np.zeros((NCORES, 128, WPS), np.int16)
    pool_pad = np.ones((NCORES, 128, WPS), bool)
    pc = pool_core[batch][porder]
    pr = pool_row[batch][porder]
    ppart = pr * 4 + subrow
    pcol = pq0[(tpos // QROWS)[porder]] + jcol
    pool_lidx[pc, ppart, pcol] = (tpos - (tpos // QROWS) * QROWS)[porder].astype(np.int16)
    pool_pad[pc, ppart, pcol] = False
    pool_maskneg = np.where(pool_pad, np.float32(-1e30), np.float32(0.0))
    out_graph = np.empty((NCORES, GPC), np.int64)
    out_graph[pool_core, pool_row] = np.arange(NGRAPHS)

    pad_factor = WTOT * 128 * NCORES / max(E, 1)
    return dict(
        tpos=tpos, node_at=node_at, BLOCKS=BLOCKS,
        lidx=lidx, WG=WG, qg0=qg0, g0=g0, WTOT=WTOT,
        pool_lidx=pool_lidx, pool_maskneg=pool_maskneg, PWQS=PWQS, pq0=pq0,
        WPS=WPS, out_graph=out_graph, pad_factor=pad_factor,
    )


def wrap_idx(vals):
    """[..., n] int16, n % 16 == 0: idx i -> [i%16, i//16], replicated x8 to
    128 partitions -> [..., 128, n/16]."""
    sh = vals.shape[:-1]
    n = vals.shape[-1]
    assert n % 16 == 0
    w = vals.reshape(*sh, n // 16, 16)
    w = np.swapaxes(w, -1, -2)
    w = np.broadcast_to(w[..., None, :, :], (*sh, 8, 16, n // 16))
    return w.reshape(*sh, 128, n // 16).copy()


def expand_a(a):
    heads, ch = a.shape
    A = np.zeros((heads * ch, heads), np.float32)
    for h in range(heads):
        A[h * ch:(h + 1) * ch, h] = a[h]
    return A


FP = mybir.dt.float32
BF = mybir.dt.bfloat16
I16 = mybir.dt.int16
ALU = mybir.AluOpType
ACTF = mybir.ActivationFunctionType
AX = mybir.AxisListType

NL = 4
GPC = 32
NEG = 0.2
NEG_OUT = 0.01
HEADS = (4, 4, 4, 1)


def build(nc, geom):
    BLOCKS = int(geom["BLOCKS"])
    NP_ = NCORES * BLOCKS * 128
    SLAB = NP_ // NCORES
    QROWS = NP_ // NQ
    WG = np.asarray(geom["WG"])            # [NGRP, NQ]
    qg0 = np.asarray(geom["qg0"])          # [NGRP, NQ+1]
    g0 = np.asarray(geom["g0"])            # [NGRP+1]
    WTOT = int(geom["WTOT"])
    PWQS = np.asarray(geom["PWQS"])        # [NQ]
    pq0 = np.asarray(geom["pq0"])          # [NQ+1]
    WPS = int(geom["WPS"])
    NGRP = BLOCKS // GS

    # ---------------- I/O ----------------
    xT = nc.declare_dram_parameter("xT", [128, SLAB], FP, isOutput=False)
    idx_in = nc.declare_dram_parameter("idx", [128, 8 * WTOT], I16, isOutput=False)
    pidx_in = nc.declare_dram_parameter("pool_idx", [128, 8 * WPS], I16, isOutput=False)
    pmask_in = nc.declare_dram_parameter("pool_maskneg", [128, WPS], FP, isOutput=False)
    wext_in = [
        nc.declare_dram_parameter(f"wext{l}", [128 if l == 0 else 64, 64 + 2 * HEADS[l]],
                                  FP, isOutput=False)
        for l in range(NL)
    ]
    bias_in = nc.declare_dram_parameter("bias", [128, NL, 64], FP, isOutput=False)
    fcW_in = nc.declare_dram_parameter("fcW", [64, 2], FP, isOutput=False)
    fcb_in = nc.declare_dram_parameter("fcb", [GPC, 2], FP, isOutput=False)
    logits_out = nc.declare_dram_parameter("logits", [GPC, 2], FP, isOutput=True)
    probas_out = nc.declare_dram_parameter("probas", [GPC, 2], FP, isOutput=True)

    with TileContext(nc) as tc, ExitStack() as ex:
        dram = ex.enter_context(tc.tile_pool(name="dram", bufs=1, space="DRAM"))
        tables = [dram.tile([NP_, 128], I16, addr_space="Shared", name=f"table{l}")
                  for l in range(NL + 1)]
        slabs = [dram.tile([SLAB, 128], I16, name=f"slab{l}") for l in range(NL + 1)]

        cpool = ex.enter_context(tc.tile_pool(name="const", bufs=1))
        gpool = ex.enter_context(tc.tile_pool(name="gath", bufs=2))
        epool = ex.enter_context(tc.tile_pool(name="esc", bufs=3))
        wtpool = ex.enter_context(tc.tile_pool(name="wt", bufs=2))
        mpool = ex.enter_context(tc.tile_pool(name="pmx", bufs=2))
        apool = ex.enter_context(tc.tile_pool(name="acc", bufs=2))
        opool = ex.enter_context(tc.tile_pool(name="outg", bufs=2))
        xpool = ex.enter_context(tc.tile_pool(name="xin", bufs=3))
        ppool = ex.enter_context(tc.tile_pool(name="psum", bufs=4, space="PSUM"))
        ppool2 = ex.enter_context(tc.tile_pool(name="psum2", bufs=2, space="PSUM"))

        # ---- constants / resident tiles ----
        wext = []
        for l in range(NL):
            t = cpool.tile([128 if l == 0 else 64, 64 + 2 * HEADS[l]], FP,
                           name=f"wext_sb{l}")
            nc.sync.dma_start(t[:], wext_in[l][:])
            wext.append(t)
        bias_sb = cpool.tile([128, NL, 64], FP)
        nc.sync.dma_start(bias_sb[:], bias_in[:])
        ident = cpool.tile([128, 128], FP)
        masks.make_identity(nc, ident[:])
        poison_t = cpool.tile([1, 4], FP)
        nc.vector.memset(poison_t[:], -120.0)
        idx_sb = cpool.tile([128, 8 * WTOT], I16)
        nc.sync.dma_start(idx_sb[:], idx_in[:])
        pidx = cpool.tile([128, 8 * WPS], I16)
        nc.sync.dma_start(pidx[:], pidx_in[:])
        pmask = cpool.tile([128, WPS], FP)
        nc.sync.dma_start(pmask[:], pmask_in[:])
        # resident local slab (this core's rows, layer-current)
        slab_sb = cpool.tile([128, BLOCKS, 128], I16)
        slab_f = slab_sb[:].bitcast(FP)           # [128, BLOCKS, 64]
        eself = [cpool.tile([128, BLOCKS, 4], FP, name=f"eself{i}") for i in range(2)]

        def matmul_to_row(l, s, lhsT_ap):
            """h_ext = lhsT.T @ wext[l] -> slab_sb[:, s, :] (bf16 h + fp32
            scores) -> DRAM slab row block."""
            H = HEADS[l]
            pm = ppool.tile([128, 64 + 2 * H], FP, tag="mm", name=f"mm_{l}_{s}")
            nc.tensor.matmul(pm[:], lhsT_ap, wext[l][:], start=True, stop=True)
            nc.scalar.copy(slab_sb[:].bitcast(BF)[:, s, 0:64], pm[:, 0:64])
            nc.scalar.copy(slab_f[:, s, 32:32 + 2 * H], pm[:, 64:64 + 2 * H])
            nc.sync.dma_start(slabs[l][s * 128:(s + 1) * 128, :], slab_sb[:, s, :])
            if s == BLOCKS - 1:
                # poison row: phantom last slab row's s_src <- -120 so padded
                # gather slots vanish in the softmax
                nc.sync.dma_start(slabs[l][SLAB - 1:SLAB, 64:72],
                                  poison_t[:].bitcast(I16))

        # ---- layer-0 matmul phase ----
        for s in range(BLOCKS):
            xt = xpool.tile([128, 128], FP, tag="xt", name=f"xt0_{s}")
            nc.sync.dma_start(xt[:], xT[:, s * 128:(s + 1) * 128])
            matmul_to_row(0, s, xt[:])

        # ---- layers ----
        for l in range(NL):
            nc.gpsimd.collective_compute(
                "AllGather", ALU.bypass,
                ins=[slabs[l][:].opt()],
                outs=[tables[l][:].opt()],
                replica_groups=[list(range(NCORES))],
            )
            H = HEADS[l]
            ch = 64 // H
            CH = 64 + H                       # wt channels: 64 feat + H den
            # self attention scores for this layer (from resident slab)
            es = eself[l % 2]
            nc.vector.tensor_tensor(es[:, :, 0:H], slab_f[:, :, 32:32 + H],
                                    slab_f[:, :, 32 + H:32 + 2 * H], ALU.add)
            nc.vector.scalar_tensor_tensor(es[:, :, 0:H], es[:, :, 0:H], NEG,
                                           es[:, :, 0:H], ALU.mult, ALU.max)
            nc.scalar.activation(es[:, :, 0:H], es[:, :, 0:H], ACTF.Exp)

            for g in range(NGRP):
                GWg = int(g0[g + 1] - g0[g])
                Gq = []
                for q in range(NQ):
                    wq = int(WG[g][q])
                    ncols = GS * wq
                    r0 = int(qg0[g][q])
                    Gt = gpool.tile([128, ncols, 128], I16, tag=f"G{q}",
                                    name=f"G_{l}_{g}_{q}")
                    nc.gpsimd.dma_gather(
                        Gt[:],
                        tables[l][q * QROWS:(q + 1) * QROWS, :],
                        idx_sb[:, 8 * int(g0[g] + r0): 8 * int(g0[g] + r0 + ncols)],
                        128 * ncols, 128 * ncols, 128,
                        single_packet=False, queue_num=q)
                    Gq.append(Gt)
                wt = wtpool.tile([128, GWg, CH], BF, tag="wt", name=f"wt_{l}_{g}")
                for q in range(NQ):
                    wq = int(WG[g][q])
                    r0 = int(qg0[g][q])
                    Gf = Gq[q][:].bitcast(FP)
                    ssrc = Gf[:, :, 32:32 + H].rearrange(
                        "p (s j) h -> p s j h", s=GS)
                    e = epool.tile([128, GS, wq, H], FP, tag="e",
                                   name=f"e_{l}_{g}_{q}")
                    nc.vector.tensor_tensor(
                        e[:], ssrc,
                        slab_f[:, GS * g:GS * (g + 1), 32 + H:32 + 2 * H]
                        .unsqueeze(2).broadcast_to([128, GS, wq, H]),
                        ALU.add)
                    nc.vector.scalar_tensor_tensor(e[:], e[:], NEG, e[:],
                                                   ALU.mult, ALU.max)
                    nc.scalar.activation(
                        wt[:, r0:r0 + GS * wq, 64:64 + H].rearrange(
                            "p (s j) h -> p s j h", s=GS),
                        e[:], ACTF.Exp)
                    # alpha-weighting multiply for this quarter's slice
                    nc.vector.tensor_tensor(
                        wt[:, r0:r0 + GS * wq, 0:64].rearrange(
                            "p c (h k) -> p c h k", h=H),
                        Gq[q][:].bitcast(BF)[:, :, 0:64].rearrange(
                            "p c (h k) -> p c h k", h=H),
                        wt[:, r0:r0 + GS * wq, 64:64 + H].unsqueeze(3)
                        .broadcast_to([128, GS * wq, H, ch]),
                        ALU.mult)
                # accumulator: [128, GS, CH, NQ+1]; slot NQ = self term
                acc = apool.tile([128, GS, CH, NQ + 1], FP, tag="acc",
                                 name=f"acc_{l}_{g}")
                nc.vector.tensor_tensor(
                    acc[:, :, 0:64, NQ].rearrange("p s (h k) -> p s h k", h=H),
                    slab_sb[:].bitcast(BF)[:, GS * g:GS * (g + 1), 0:64].rearrange(
                        "p s (h k) -> p s h k", h=H),
                    es[:, GS * g:GS * (g + 1), 0:H].unsqueeze(3)
                    .broadcast_to([128, GS, H, ch]),
                    ALU.mult)
                nc.vector.tensor_copy(acc[:, :, 64:64 + H, NQ],
                                      es[:, GS * g:GS * (g + 1), 0:H])
                for q in range(NQ):
                    wq = int(WG[g][q])
                    r0 = int(qg0[g][q])
                    nc.vector.tensor_reduce(
                        acc[:, :, :, q],
                        wt[:, r0:r0 + GS * wq, :].rearrange(
                            "p (s j) c -> p s c j", s=GS),
                        axis=AX.X, op=ALU.add)
                tot = apool.tile([128, GS, CH], FP, tag="tot", name=f"tot_{l}_{g}")
                nc.vector.tensor_reduce(tot[:], acc[:], axis=AX.X, op=ALU.add)
                rden = apool.tile([128, GS, H], FP, tag="rd", name=f"rd_{l}_{g}")
                nc.vector.reciprocal(rden[:], tot[:, :, 64:64 + H])
                outg = opool.tile([128, GS, 64], FP, tag="outg", name=f"og_{l}_{g}")
                nc.vector.tensor_tensor(
                    outg[:].rearrange("p s (h k) -> p s h k", h=H),
                    tot[:, :, 0:64].rearrange("p s (h k) -> p s h k", h=H),
                    rden[:].unsqueeze(3).broadcast_to([128, GS, H, ch]),
                    ALU.mult)
                nc.vector.tensor_tensor(
                    outg[:], outg[:],
                    bias_sb[:, l, :].unsqueeze(1).broadcast_to([128, GS, 64]),
                    ALU.add)
                nc.vector.scalar_tensor_tensor(outg[:], outg[:], NEG_OUT, outg[:],
                                               ALU.mult, ALU.max)
                if l < NL - 1:
                    # one [128,128] transpose covers both slots of the group
                    pt = ppool2.tile([128, 128], FP, tag="tp", name=f"tp_{l}_{g}")
                    nc.tensor.transpose(
                        pt[:], outg[:].rearrange("p s f -> p (s f)"), ident[:])
                    xtn = xpool.tile([64, GS, 128], FP, tag="xtn", name=f"xtn_{l}_{g}")
                    for si in range(GS):
                        nc.scalar.copy(xtn[:, si, :], pt[64 * si:64 * (si + 1), :])
                    for si in range(GS):
                        matmul_to_row(l + 1, GS * g + si, xtn[:, si, :])
                else:
                    for si in range(GS):
                        s = GS * g + si
                        rowf = slab_f[:, s, :]
                        nc.vector.tensor_copy(rowf, outg[:, si, :])
                        nc.sync.dma_start(slabs[NL][s * 128:(s + 1) * 128, :],
                                          slab_sb[:, s, :])

        # ---- final AllGather (x_final fp32 rows) ----
        nc.gpsimd.collective_compute(
            "AllGather", ALU.bypass,
            ins=[slabs[NL][:].opt()],
            outs=[tables[NL][:].opt()],
            replica_groups=[list(range(NCORES))],
        )

        # ---- pooling ----
        pooled = cpool.tile([128, 64], FP)
        first = True
        PCH = 16
        for q in range(NQ):
            for k0 in range(0, int(PWQS[q]), PCH):
                wq = min(PCH, int(PWQS[q]) - k0)
                c0 = int(pq0[q]) + k0
                PG = gpool.tile([128, wq, 128], I16, tag=f"G{q}",
                                name=f"PG_{q}_{k0}")
                nc.gpsimd.dma_gather(
                    PG[:], tables[NL][q * QROWS:(q + 1) * QROWS, :],
                    pidx[:, 8 * c0: 8 * (c0 + wq)],
                    128 * wq, 128 * wq, 128,
                    single_packet=False, queue_num=q)
                PGf = PG[:].bitcast(FP)                    # [128, wq, 64]
                pm = mpool.tile([128, wq, 64], FP, tag="pm", name=f"pm_{q}_{k0}")
                nc.vector.tensor_tensor(
                    pm[:], PGf,
                    pmask[:, c0:c0 + wq].unsqueeze(2).broadcast_to([128, wq, 64]),
                    ALU.add)
                red = mpool.tile([128, 64], FP, tag="red", name=f"red_{q}_{k0}")
                nc.vector.tensor_reduce(red[:], pm[:].rearrange("p w f -> p f w"),
                                        axis=AX.X, op=ALU.max)
                if first:
                    nc.vector.tensor_copy(pooled[:], red[:])
                    first = False
                else:
                    nc.vector.tensor_tensor(pooled[:], pooled[:], red[:], ALU.max)
        # transpose + fold the 4 subrows per graph via strided max-reduce
        ptp = ppool2.tile([64, 128], FP, tag="tp", name="pool_tp")
        nc.tensor.transpose(ptp[:], pooled[:], ident[:])
        ptps = cpool.tile([64, 128], FP)
        nc.scalar.copy(ptps[:], ptp[:])
        pooledT = cpool.tile([64, GPC], FP)
        nc.vector.tensor_reduce(
            pooledT[:], ptps[:].rearrange("p (g r) -> p g r", r=4),
            axis=AX.X, op=ALU.max)
        # FC + bias + softmax
        fcW = cpool.tile([64, 2], FP)
        nc.sync.dma_start(fcW[:], fcW_in[:])
        fcb = cpool.tile([GPC, 2], FP)
        nc.sync.dma_start(fcb[:], fcb_in[:])
        plog = ppool.tile([GPC, 2], FP, tag="mm", name="logits_mm")
        nc.tensor.matmul(plog[:], pooledT[:], fcW[:], start=True, stop=True)
        logits = cpool.tile([GPC, 2], FP)
        nc.vector.tensor_tensor(logits[:], plog[:], fcb[:], ALU.add)
        nc.sync.dma_start(logits_out[:], logits[:])
        m = cpool.tile([GPC, 1], FP)
        nc.vector.tensor_reduce(m[:], logits[:], axis=AX.X, op=ALU.max)
        z = cpool.tile([GPC, 2], FP)
        nc.vector.tensor_tensor(z[:], logits[:], m[:].broadcast_to([GPC, 2]),
                                ALU.subtract)
        ez = cpool.tile([GPC, 2], FP)
        nc.scalar.activation(ez[:], z[:], ACTF.Exp)
        den2 = cpool.tile([GPC, 1], FP)
        nc.vector.tensor_reduce(den2[:], ez[:], axis=AX.X, op=ALU.add)
        rden2 = cpool.tile([GPC, 1], FP)
        nc.vector.reciprocal(rden2[:], den2[:])
        probas = cpool.tile([GPC, 2], FP)
        nc.vector.tensor_tensor(probas[:], ez[:], rden2[:].broadcast_to([GPC, 2]),
                                ALU.mult)
        nc.sync.dma_start(probas_out[:], probas[:])
    return nc


def make_inputs(P, inp):
    """Per-core in_maps from preprocess() result P and problem inputs."""
    BLOCKS = int(P["BLOCKS"])
    SLAB = BLOCKS * 128
    x = np.asarray(inp["x"], np.float32)
    F = x.shape[1]
    wext_np = []
    for l in range(NL):
        Wl = np.asarray(inp[f"W{l+1}"], np.float32)
        As = expand_a(np.asarray(inp[f"a{l+1}s"], np.float32))
        Ad = expand_a(np.asarray(inp[f"a{l+1}d"], np.float32))
        wext_np.append(np.concatenate([Wl, Wl @ As, Wl @ Ad], axis=1))
    bias_np = np.stack([np.asarray(inp[f"b{l+1}"], np.float32) for l in range(NL)])
    bias_rep = np.tile(bias_np[None], (128, 1, 1))
    fcW = np.asarray(inp["fcW"], np.float32)
    fcb = np.tile(np.asarray(inp["fcb"], np.float32)[None, :], (GPC, 1))

    WG, qg0, g0 = P["WG"], P["qg0"], P["g0"]
    NGRP = BLOCKS // GS
    regions = []
    for g in range(NGRP):
        for q in range(NQ):
            c0 = int(g0[g] + qg0[g][q])
            regions.append((c0, GS * int(WG[g][q])))
    pregions = [(int(P["pq0"][q]), int(P["PWQS"][q])) for q in range(NQ)]

    def build_idx(lidx_c, regs):
        parts = []
        for c0, ncols in regs:
            stream = lidx_c[:, c0:c0 + ncols].T.reshape(1, -1)   # col-major
            parts.append(wrap_idx(stream)[0])
        return np.concatenate(parts, axis=1).astype(np.int16)

    in_maps = []
    for c in range(NCORES):
        nodes = P["node_at"][c]
        xs = np.zeros((SLAB, F), np.float32)
        valid = nodes >= 0
        xs[valid] = x[nodes[valid]]
        m = {
            "xT": np.ascontiguousarray(xs.T),
            "idx": build_idx(P["lidx"][c], regions),
            "pool_idx": build_idx(P["pool_lidx"][c], pregions),
            "pool_maskneg": P["pool_maskneg"][c].astype(np.float32),
            "bias": bias_rep, "fcW": fcW, "fcb": fcb,
        }
        for l in range(NL):
            m[f"wext{l}"] = wext_np[l]
        in_maps.append(m)
    return in_maps


def _run(inputs, trace=False):
    inp = {k: np.asarray(v) for k, v in inputs.items()}
    P = preprocess(inp['edge_index'], inp['batch'], N=100000, BLOCKS=98,
                   NGRAPHS=256)
    print(f"pad_factor: {P['pad_factor']:.3f}, WTOT={P['WTOT']}")
    in_maps = make_inputs(P, inp)
    nc = bacc.Bacc("TRN2", num_swdge_queues=4)
    build(nc, P)
    nc.compile()
    res = run_bass_kernel_spmd(nc, in_maps, list(range(NCORES)), trace=trace)
    global LAST_RES
    LAST_RES = res
    if trace and res.instructions_and_trace:
        print(f"trace path: {res.instructions_and_trace[1]}")
    logits = np.zeros((256, 2), np.float32)
    probas = np.zeros((256, 2), np.float32)
    for c in range(NCORES):
        lg = res.results[c]["logits"]
        pb = res.results[c]["probas"]
        for r in range(GPC):
            g = P["out_graph"][c, r]
            logits[g] = lg[r]
            probas[g] = pb[r]
    return logits, probas, res.exec_time_ns


def kernel(**inputs):
    logits, probas, _ = _run(inputs, trace=False)
    return logits, probas
